# revision 16
# baseline (speedup 1.0000x reference)
"""DiversityDensity kernel for 8x Trainium2 NeuronCores.

Math: for each row u of U_z:
    dens(u)  = -0.5*||u||^2 - 0.5*NZ*log(2*pi)
    div(u)   = min_l ||u - l||_2  over rows l of L_z
    dd       = exp(dens + log(div + eps)); dd = (dd - min dd) / (max dd + eps)

Device computes m(u) = min_l (||l||^2 - 2 u.l) via a K=34 fp16 matmul
(32 features + split ||l||^2 hi/lo rows against ones) streamed over L in
512-row chunks, with a running elementwise min over the PSUM tiles.
Layout: L-rows on output partitions (M), u on the free dim (N), so the
final reduce is a cheap cross-partition min done on host after gather.
K=34 <= 64 allows 2-way PE row-group packing (two chunks matmul
concurrently in array rows 0-63 / 64-127) — important because the PE
runs at 1.2 GHz (cold HAM clock) in this environment.

PSUM drain (the throughput bound): matmuls write fp16 into PSUM, so the
DVE tensor_tensor(min) can stream 2 packed fp16 elements per cycle
(2x_1P mode) straight from PSUM into an SBUF fp16 accumulator.  An
optional ScalarE-copy route exists as a fallback/balancer.
d(u)^2 = ||u||^2 + m(u); sqrt/exp/normalize on host (O(N_U) work).

Sharding: U_z rows split 8 ways (512 rows/core); L_z (8 MB) replicated.
"""

import numpy as np

N_U, N_L, NZ = 4096, 65536, 32
CORES = 8
SHARD = N_U // CORES  # 512
K = NZ + 2  # 34: 32 features + c_hi + c_lo rows
NCHUNK = 512  # L-rows per group (4 matmuls of M=128)
GROUPS = N_L // NCHUNK  # 128
LOG_2PI = float(np.log(2.0 * np.pi))
EPS = 1e-18

# Per-group drain route: A = DVE tensor_tensor(min) fp32 from PSUM,
# B = ScalarE copy to SBUF fp16 + DVE min at 2x.  Balances DVE vs ACT.
N_A = 35

TRACE = False
LAST = {}

_CACHE = {}


def _route_a(g: int) -> bool:
    return (g + 1) * N_A // GROUPS > g * N_A // GROUPS


def _build():
    import concourse.bass as bass  # noqa: F401
    import concourse.tile as tile
    from concourse import bacc, mybir

    f32 = mybir.dt.float32
    f16 = mybir.dt.float16
    MIN = mybir.AluOpType.min

    nc = bacc.Bacc(
        "TRN2", target_bir_lowering=False, debug=False, num_devices=CORES
    )
    ut_d = nc.declare_dram_parameter("ut", [K, SHARD], f16, isOutput=False)
    lt_d = nc.declare_dram_parameter("lt", [GROUPS, K, NCHUNK], f16, isOutput=False)
    out_d = nc.declare_dram_parameter("partmin", [128, SHARD], f32, isOutput=True)

    with tile.TileContext(nc) as tc:
        with (
            tc.tile_pool(name="const", bufs=1) as cpool,
            tc.tile_pool(name="ltp", bufs=6) as ltpool,
            tc.tile_pool(name="accp", bufs=1) as accpool,
            tc.tile_pool(name="s16p", bufs=6) as s16pool,
            tc.tile_pool(name="tailp", bufs=1) as tailpool,
            tc.tile_pool(name="psum", bufs=2, space="PSUM") as pspool,
        ):
            # rhs (U^T in fp16, plus two ones rows) at both PE row-group bases
            ut_t = cpool.tile([64 + K, SHARD], f16)
            nc.sync.dma_start(ut_t[0:K, :], ut_d[:, :])
            nc.sync.dma_start(ut_t[64 : 64 + K, :], ut_d[:, :])

            acc32 = accpool.tile([128, 2048], f32)
            acc16 = accpool.tile([128, 2048], f16)
            nc.vector.memset(acc32[:], 3.0e38)
            nc.vector.memset(acc16[:], 60000.0)

            # HAM warmup: ~5us of back-to-back matmuls while the first lt
            # DMAs land, so the PE clock ungates to 2.4 GHz (K=8/8) before
            # the main loop.  Results are scratch (overwritten slot).
            wu = pspool.tile([128, 2048], f32, tag="ps")
            for _ in range(12):
                nc.tensor.matmul(
                    wu[:, 0:512],
                    lhsT=ut_t[0:K, 0:128],
                    rhs=ut_t[0:K, :],
                    start=True,
                    stop=True,
                )

            for g in range(GROUPS):
                base = 0 if (g % 2 == 0) else 64
                lt_t = ltpool.tile([64 + K, NCHUNK], f16, tag="lt")
                nc.sync.dma_start(lt_t[base : base + K, :], lt_d[g])

                ps = pspool.tile([128, 2048], f32, tag="ps")
                for c in range(4):
                    nc.tensor.matmul(
                        ps[:, c * 512 : (c + 1) * 512],
                        lhsT=lt_t[base : base + K, c * 128 : (c + 1) * 128],
                        rhs=ut_t[base : base + K, :],
                        start=True,
                        stop=True,
                    )
                if _route_a(g):
                    nc.vector.tensor_tensor(acc32[:], acc32[:], ps[:], MIN)
                else:
                    s16 = s16pool.tile([128, 2048], f16, tag="s16")
                    nc.scalar.copy(s16[:], ps[:])
                    nc.vector.tensor_tensor(acc16[:], acc16[:], s16[:], MIN)

            # Tail: fold the 4 sub-chunk columns together, merge routes.
            t16a = tailpool.tile([128, 1024], f16)
            nc.vector.tensor_tensor(t16a[:], acc16[:, 0:1024], acc16[:, 1024:2048], MIN)
            t16b = tailpool.tile([128, 512], f16)
            nc.vector.tensor_tensor(t16b[:], t16a[:, 0:512], t16a[:, 512:1024], MIN)
            t16c = tailpool.tile([128, 512], f32)
            nc.vector.tensor_copy(t16c[:], t16b[:])

            t32a = tailpool.tile([128, 1024], f32)
            nc.vector.tensor_tensor(t32a[:], acc32[:, 0:1024], acc32[:, 1024:2048], MIN)
            t32b = tailpool.tile([128, 512], f32)
            nc.vector.tensor_tensor(t32b[:], t32a[:, 0:512], t32a[:, 512:1024], MIN)

            res = tailpool.tile([128, 512], f32)
            nc.vector.tensor_tensor(res[:], t32b[:], t16c[:], MIN)
            nc.sync.dma_start(out_d[:, :], res[:])

    nc.compile()
    return nc


def _get_nc():
    if "nc" not in _CACHE:
        _CACHE["nc"] = _build()
    return _CACHE["nc"]


def kernel(pred: np.ndarray, U_z: np.ndarray, L_z: np.ndarray) -> np.ndarray:
    from concourse.bass_utils import run_bass_kernel_spmd

    f16 = np.float16
    U = np.asarray(U_z, dtype=np.float32)
    L = np.asarray(L_z, dtype=np.float32)

    # Host prep: augmented, transposed fp16 operands.
    # Contraction rows: L side [-2*L^T (32); c_hi; c_lo] vs
    #                   U side [U^T    (32);    1;    1]
    c = np.einsum("ij,ij->i", L.astype(np.float64), L.astype(np.float64))
    c_hi = c.astype(f16)
    c_lo = (c - c_hi.astype(np.float64)).astype(f16)
    lt = np.empty((K, N_L), dtype=f16)
    lt[0:NZ] = (-2.0 * L.T).astype(f16)
    lt[NZ] = c_hi
    lt[NZ + 1] = c_lo
    # Block per group so each DMA reads one contiguous [K, NCHUNK] slab.
    lt_blocked = np.ascontiguousarray(
        lt.reshape(K, GROUPS, NCHUNK).transpose(1, 0, 2)
    )

    in_maps = []
    for i in range(CORES):
        ut = np.empty((K, SHARD), dtype=f16)
        ut[0:NZ] = U[i * SHARD : (i + 1) * SHARD].T.astype(f16)
        ut[NZ] = f16(1.0)
        ut[NZ + 1] = f16(1.0)
        in_maps.append({"ut": np.ascontiguousarray(ut), "lt": lt_blocked})

    nc = _get_nc()
    kwargs = {}
    if TRACE:
        import os
        import shutil

        tdir = "/root/problem/trace_out"
        shutil.rmtree(tdir, ignore_errors=True)
        os.makedirs(tdir, exist_ok=True)
        kwargs["tmpdir"] = tdir
    res = run_bass_kernel_spmd(nc, in_maps, list(range(CORES)), trace=TRACE, **kwargs)
    LAST["exec_time_ns"] = res.exec_time_ns
    LAST["results"] = res

    # Gather: cross-partition min on host, then the cheap scalar tail.
    minval = np.empty(N_U, dtype=np.float32)
    for i in range(CORES):
        pm = res.results[i]["partmin"]  # [128, SHARD]
        minval[i * SHARD : (i + 1) * SHARD] = pm.min(axis=0)

    u_sq = np.einsum("ij,ij->i", U, U, dtype=np.float32)
    d2 = np.maximum(u_sq + minval, 0.0).astype(np.float32)
    div = np.sqrt(d2)
    dens = (-0.5 * u_sq - 0.5 * NZ * LOG_2PI).astype(np.float32)
    dd = np.exp(dens + np.log(div + EPS)).astype(np.float32)
    dd = dd - dd.min()
    dd = dd / (dd.max() + np.float32(EPS))
    return dd.astype(np.float32)
